# revision 5
# baseline (speedup 1.0000x reference)
"""GCN LinearEncoder kernel for Trainium2 (8 NeuronCores, Bass/Tile).

Computes out = D^-1/2 (A+I) D^-1/2 (x W^T) + b  for a 100k-node / 3.2M-edge
random graph, D_in = D_out = 128.

Strategy (1D data parallel over destination nodes, per sharding hint):
  * Host: add self-loops, compute deg^-1/2 (a by-product of routing edges to
    their destination-owner core), and bucket edges by
    (dst core, dst window of 128 rows, src chunk of 25000 rows).
    Each (window, chunk) run is padded to a multiple of 128 edge slots; for
    every slot we ship (src_local int16, dst_offset f32, g_src f32) packed
    into one int32 metadata tensor. Windows are processed in groups of 2 so
    one dma_gather per src-chunk covers both windows (fewer, bigger gathers).
  * Device (identical program on all 8 cores, no collectives):
      per window group:
        - DMA packed metadata -> SBUF
        - 4 dma_gather (custom InstDMAGatherAnt, int16 wrapped indices):
          x[src] rows -> SBUF [128 edges, T, 128] (512B per edge row)
        - per 128-edge tile: one DVE tensor_scalar builds the weighted
          selection matrix  oh[e, j] = (iota[j] == dstl[e]) * g_src[e];
          one PE matmul accumulates aggT[i, dst] += msgs^T @ oh in PSUM
        - flush per window: aggT -> SBUF; out_psum[dst, f] = aggT^T @ W^T
          (second matmul — no transposes anywhere); scale rows by g_dst,
          add bias, DMA 128 output rows to DRAM.

The gather is the roofline term: ~230MB of random 512B reads per core.
DVE selection builds, PE matmuls and flushes hide under it.
"""

import os

os.environ.setdefault("JAX_PLATFORMS", "axon")

import numpy as np

P = 128          # partitions / feature dim / window size
CHUNK = 25000    # src rows per gather table chunk (must be < 32768 for int16)
WG = 2           # windows per group (one gather per chunk spans the group)


# ---------------------------------------------------------------- host side


def _route(x, src, dst, n_cores, chunk):
    """Bucket edges by (core, window, chunk); pack per-core device metadata."""
    n = x.shape[0]
    rows_per_core = (n + n_cores - 1) // n_cores
    nw = (rows_per_core + P - 1) // P
    if nw % WG:
        nw += WG - nw % WG  # keep groups even (extra windows are all-pad)
    nch = (n + chunk - 1) // chunk

    loops = np.arange(n, dtype=np.int64)
    src = np.concatenate([src, loops])
    dst = np.concatenate([dst, loops])
    deg = np.bincount(dst, minlength=n)
    g = np.zeros(n, np.float32)
    nz = deg > 0
    g[nz] = (1.0 / np.sqrt(deg[nz].astype(np.float64))).astype(np.float32)

    core = dst // rows_per_core
    dloc = dst - core * rows_per_core
    wl = dloc >> 7
    ch = src // chunk
    key = ((core * nw + wl) * nch + ch).astype(np.int64)
    order = np.argsort(key, kind="stable")
    src_s = src[order]
    key_s = key[order]
    dwin_s = (dloc[order] & 127).astype(np.float32)
    srcloc_s = (src_s - (src_s // chunk) * chunk).astype(np.int16)
    gsrc_s = g[src_s]
    core_s = core[order]

    counts = np.bincount(key_s, minlength=n_cores * nw * nch).reshape(
        n_cores, nw, nch
    )
    tc_tiles = np.maximum(1, -(-counts.max(axis=0) // P))  # [nw, nch]

    # group/tile geometry (shared across cores)
    n_grp = nw // WG
    t_in_g = tc_tiles.reshape(n_grp, WG, nch).transpose(0, 2, 1)  # [g, c, wi]
    tg = t_in_g.reshape(n_grp, -1).sum(axis=1)  # tiles per group
    # tile offset of run (w, c) within its group, (c, w)-ordered
    run_off = np.zeros((nw, nch), np.int64)
    for gi in range(n_grp):
        off = 0
        for c in range(nch):
            for wi in range(WG):
                run_off[gi * WG + wi, c] = off
                off += tc_tiles[gi * WG + wi, c]
    moff = np.zeros(n_grp + 1, np.int64)  # int32-col offset of each group
    moff[1:] = np.cumsum(6 * tg)
    w_total = int(moff[-1])

    meta = np.zeros((n_cores, P, w_total), np.int32)
    # init dstl regions to 128.0f (pad -> zero one-hot row)
    pad_bits = np.float32(128.0).view(np.int32)
    dstl_cols = np.zeros(w_total, bool)
    idx_cols = np.zeros(w_total, bool)
    for gi in range(n_grp):
        t = int(tg[gi])
        dstl_cols[moff[gi] : moff[gi] + t] = True
        idx_cols[moff[gi] + 2 * t : moff[gi] + 6 * t] = True
    meta[:, :, dstl_cols] = pad_bits

    # per-edge positions
    edge_start = np.concatenate([[0], np.cumsum(counts.reshape(-1))])
    rank = np.arange(len(src_s)) - edge_start[key_s]
    w_of = (key_s // nch) % nw
    gi_of = w_of // WG
    tile_in_g = run_off[w_of, key_s % nch] + (rank >> 7)
    col = moff[gi_of] + tile_in_g
    tgg = tg[gi_of]
    p_of = rank & 127

    mflat = meta.reshape(n_cores, -1)
    rowbase = p_of * w_total
    cflat = core_s
    mflat[cflat, rowbase + col] = dwin_s.view(np.int32)
    mflat[cflat, rowbase + tgg + col] = gsrc_s.view(np.int32)

    # int16 indices: element i of a run -> [i % 16, i // 16] in the run block,
    # block replicated across the 8 groups of 16 partitions.
    meta16 = meta.view(np.int16).reshape(n_cores, P, 2 * w_total)
    base16 = (moff[gi_of] + 2 * tgg + 4 * run_off[w_of, key_s % nch]) * 2
    m16flat = meta16.reshape(n_cores, -1)
    m16flat[cflat, (rank & 15) * (2 * w_total) + base16 + (rank >> 4)] = srcloc_s
    # replicate idx regions from partitions 0-15 to 16-127
    idx16 = np.zeros(2 * w_total, bool)
    idx16[0::2] = idx_cols
    idx16[1::2] = idx_cols
    blk = meta16[:, :16, :][:, :, idx16]
    meta16[:, :, idx16] = np.tile(blk, (1, 8, 1))

    # g_dst [cores, 128, nw]
    gg = np.zeros((n_cores, nw * P), np.float32)
    gg[:, :rows_per_core] = np.pad(
        g, (0, n_cores * rows_per_core - n)
    ).reshape(n_cores, rows_per_core)
    gdst = np.ascontiguousarray(gg.reshape(n_cores, nw, P).transpose(0, 2, 1))

    return dict(
        meta=meta, gdst=gdst, nw=nw, nch=nch, n_grp=n_grp, tg=tg,
        tc=tc_tiles, run_off=run_off, moff=moff, w_total=w_total,
        rows_per_core=rows_per_core,
    )


def _consts(weight, bias):
    wt = np.ascontiguousarray(np.asarray(weight, np.float32).T)
    iota = np.ascontiguousarray(np.tile(np.arange(P, dtype=np.float32), (P, 1)))
    bias_rep = np.ascontiguousarray(np.tile(np.asarray(bias, np.float32), (P, 1)))
    return wt, iota, bias_rep


# ---------------------------------------------------------------- device side


def _build_program(n_nodes, d, r, n_cores, chunk,
                   gath_bufs=3, oh_bufs=8, meta_bufs=3):
    from concourse import bacc, mybir
    from concourse.tile import TileContext

    f32, i32, i16 = mybir.dt.float32, mybir.dt.int32, mybir.dt.int16
    eq, mul = mybir.AluOpType.is_equal, mybir.AluOpType.mult

    nw, nch, n_grp = r["nw"], r["nch"], r["n_grp"]
    tg, tc, run_off, moff = r["tg"], r["tc"], r["run_off"], r["moff"]
    w_total, rows_per_core = r["w_total"], r["rows_per_core"]
    tg_max = int(tg.max())

    nc = bacc.Bacc(
        "TRN2", target_bir_lowering=False, debug=False, num_devices=n_cores
    )
    x_d = nc.dram_tensor("x", [n_nodes, d], f32, kind="ExternalInput").ap()
    meta_d = nc.dram_tensor("meta", [P, w_total], i32, kind="ExternalInput").ap()
    gdst_d = nc.dram_tensor("gdst", [P, nw], f32, kind="ExternalInput").ap()
    wt_d = nc.dram_tensor("wt", [d, d], f32, kind="ExternalInput").ap()
    iota_d = nc.dram_tensor("iota", [P, P], f32, kind="ExternalInput").ap()
    bias_d = nc.dram_tensor("bias_rep", [P, d], f32, kind="ExternalInput").ap()
    out_d = nc.dram_tensor(
        "out", [rows_per_core, d], f32, kind="ExternalOutput"
    ).ap()

    with TileContext(nc) as tc_ctx:
        with (
            tc_ctx.tile_pool(name="const", bufs=1) as cpool,
            tc_ctx.tile_pool(name="meta", bufs=meta_bufs) as mpool,
            tc_ctx.tile_pool(name="gath", bufs=gath_bufs) as gpool,
            tc_ctx.tile_pool(name="oh", bufs=oh_bufs) as ohpool,
            tc_ctx.tile_pool(name="fl", bufs=3) as flpool,
            tc_ctx.tile_pool(name="agg_ps", bufs=3, space="PSUM") as apool,
            tc_ctx.tile_pool(name="out_ps", bufs=2, space="PSUM") as opool,
        ):
            iota_sb = cpool.tile([P, P], f32)
            nc.sync.dma_start(out=iota_sb[:], in_=iota_d[:, :])
            wt_sb = cpool.tile([d, d], f32)
            nc.sync.dma_start(out=wt_sb[:], in_=wt_d[:, :])
            bias_sb = cpool.tile([P, d], f32)
            nc.sync.dma_start(out=bias_sb[:], in_=bias_d[:, :])
            gdst_sb = cpool.tile([P, nw], f32)
            nc.sync.dma_start(out=gdst_sb[:], in_=gdst_d[:, :])

            for gi in range(n_grp):
                t_g = int(tg[gi])
                mo = int(moff[gi])
                meta_t = mpool.tile([P, 6 * tg_max], i32, tag="meta")
                nc.sync.dma_start(
                    out=meta_t[:, : 6 * t_g], in_=meta_d[:, mo : mo + 6 * t_g]
                )
                gath_t = gpool.tile([P, tg_max, P], f32, tag="gath")
                ws = [gi * WG + wi for wi in range(WG)]
                gmax = 8  # tiles per dma_gather (1024-index HW limit)
                for c in range(nch):
                    t0 = int(run_off[ws[0], c])
                    ntile = int(sum(tc[w, c] for w in ws))
                    rows_c = min(chunk, n_nodes - c * chunk)
                    for s0 in range(t0, t0 + ntile, gmax):
                        sn = min(gmax, t0 + ntile - s0)
                        ni = sn * P
                        idx_ap = meta_t[
                            :, 2 * t_g + 4 * s0 : 2 * t_g + 4 * (s0 + sn)
                        ].bitcast(i16)
                        nc.gpsimd.dma_gather(
                            gath_t[:, s0 : s0 + sn, :],
                            x_d[c * chunk : c * chunk + rows_c, :],
                            idx_ap,
                            ni,
                            ni,
                            P,
                        )
                for w in ws:
                    tiles = []
                    for c in range(nch):
                        t0 = int(run_off[w, c])
                        tiles.extend(range(t0, t0 + int(tc[w, c])))
                    agg_ps = apool.tile([P, P], f32)
                    for k, t in enumerate(tiles):
                        oh_t = ohpool.tile([P, P], f32)
                        nc.vector.tensor_scalar(
                            oh_t[:],
                            iota_sb[:],
                            meta_t[:, t : t + 1].bitcast(f32),
                            meta_t[:, t_g + t : t_g + t + 1].bitcast(f32),
                            eq,
                            mul,
                        )
                        nc.tensor.matmul(
                            out=agg_ps[:],
                            lhsT=gath_t[:, t, :],
                            rhs=oh_t[:],
                            start=(k == 0),
                            stop=(k == len(tiles) - 1),
                        )
                    aggt_sb = flpool.tile([P, P], f32, tag="aggT")
                    nc.vector.tensor_copy(out=aggt_sb[:], in_=agg_ps[:])
                    out_ps = opool.tile([P, d], f32)
                    nc.tensor.matmul(
                        out=out_ps[:], lhsT=aggt_sb[:], rhs=wt_sb[:],
                        start=True, stop=True,
                    )
                    rows = min(P, rows_per_core - w * P)
                    if rows <= 0:
                        continue
                    out_sb = flpool.tile([P, d], f32, tag="out")
                    nc.vector.tensor_scalar(
                        out_sb[:], out_ps[:], gdst_sb[:, w : w + 1], None, mul
                    )
                    nc.vector.tensor_add(
                        out=out_sb[:], in0=out_sb[:], in1=bias_sb[:]
                    )
                    nc.sync.dma_start(
                        out=out_d[w * P : w * P + rows, :], in_=out_sb[:rows, :]
                    )

    nc.compile()
    return nc


# ---------------------------------------------------------------- entry point

_CACHE = {}


def _prepare(x, edge_index, weight, bias, n_cores=8, chunk=CHUNK):
    x = np.ascontiguousarray(np.asarray(x, dtype=np.float32))
    ei = np.asarray(edge_index)
    r = _route(x, ei[0].astype(np.int64), ei[1].astype(np.int64), n_cores, chunk)
    key = (x.shape, r["nw"], r["nch"], r["w_total"],
           tuple(int(v) for v in r["tg"]))
    if key not in _CACHE:
        _CACHE[key] = _build_program(
            x.shape[0], x.shape[1], r, n_cores, chunk
        )
    return x, r, _CACHE[key]


def kernel(x, edge_index, weight, bias, _trace=False, _chunk=CHUNK):
    from concourse.bass_utils import run_bass_kernel_spmd

    n_cores = 8
    x, r, nc = _prepare(x, edge_index, weight, bias, n_cores, chunk=_chunk)
    wt, iota, bias_rep = _consts(weight, bias)
    in_maps = [
        dict(
            x=x,
            meta=np.ascontiguousarray(r["meta"][c]),
            gdst=np.ascontiguousarray(r["gdst"][c]),
            wt=wt,
            iota=iota,
            bias_rep=bias_rep,
        )
        for c in range(n_cores)
    ]
    res = run_bass_kernel_spmd(
        nc, in_maps, core_ids=list(range(n_cores)), trace=_trace
    )
    n = x.shape[0]
    out = np.concatenate([res.results[c]["out"] for c in range(n_cores)])[:n]
    if _trace:
        kernel.last_results = res
    return out, 0
